# revision 1
# baseline (speedup 1.0000x reference)
"""Trainium2 Bass kernel for 3x3 conv (stride 1, pad 1) + bias.

x [32, 64, 224, 224] f32, weight [128, 64, 3, 3] f32, bias [128] f32
-> out [32, 128, 224, 224] f32.

Data-parallel over 8 NeuronCores: core c computes samples [4c, 4c+4).

Per-core scheme (v2, all dims hardcoded):
- x is zero-padded to [4, 64, 226, 226] on the host, so every strip DMA is
  fully contiguous and all matmul windows are uniform (no edge cases).
- float32r matmuls (1 cycle/row on PE, ~13-bit mantissa).
- K=128 packing: SBUF strip holds padded x rows on partitions 0-63 (top)
  and the same rows shifted one row down on partitions 64-127 (bottom,
  built by one SBUF->SBUF DMA per strip). One K=128 matmul computes the
  kh=0 AND kh=1 contributions together (weights for the two kh stacked on
  the partition halves); kh=2 is a K=64 top-half matmul.
  6 matmuls per 2-output-row block instead of 9.
- Strips of 56 output rows (58 padded input rows), double buffered.
  in/dup DMAs ride the ACT HWDGE ring, store DMAs the SP HWDGE ring, so
  input and output transfers overlap.
- PSUM accumulation; ScalarE evacuates psum->SBUF fused with the bias add;
  store tiles batch 8 output rows so each store DMA moves ~0.9 MB.
- weight is transposed/stacked and bias reshaped on host (numpy).
"""
import numpy as np

import concourse.bass as bass
import concourse.mybir as mybir
import concourse.tile as tile
from concourse import bacc
from concourse.bass_utils import run_bass_kernel_spmd
from concourse._compat import axon_active

N_CORES = 8
S = 4                 # samples per core
IC, OC, H, W = 64, 128, 224, 224
HP, WP = H + 2, W + 2  # padded input dims (226)
QROWS = 56            # output rows per strip
SROWS = QROWS + 2     # 58 padded input rows per strip
NQ = H // QROWS       # 4 strips per sample
BLK = 2               # output rows per block
OBLK = 8              # output rows per store tile (4 blocks)

F32R = mybir.dt.float32r
F32 = mybir.dt.float32


def build_module(repeat=1):
    nc = bacc.Bacc("TRN2", target_bir_lowering=False, debug=not axon_active(),
                   enable_asserts=True, num_devices=N_CORES)
    xs = nc.dram_tensor("xs", [S, IC, HP, WP], F32R, kind="ExternalInput").ap()
    # wpair[0:64, kw*128+oc] = w[oc, ic, kh=0, kw]; [64:128, ...] = kh=1
    wpair = nc.dram_tensor("wpair", [2 * IC, 3 * OC], F32R, kind="ExternalInput").ap()
    # wk2[ic, kw*128+oc] = w[oc, ic, kh=2, kw]
    wk2 = nc.dram_tensor("wk2", [IC, 3 * OC], F32R, kind="ExternalInput").ap()
    bias = nc.dram_tensor("bias", [OC, 1], F32, kind="ExternalInput").ap()
    out = nc.dram_tensor("out", [S, OC, H, W], F32, kind="ExternalOutput").ap()

    with tile.TileContext(nc) as tc:
        with tc.tile_pool(name="wp", bufs=1) as wp, \
             tc.tile_pool(name="xp", bufs=2) as xp, \
             tc.tile_pool(name="op", bufs=3) as op, \
             tc.tile_pool(name="pp", bufs=6, space="PSUM") as pp:
            wpt = wp.tile([2 * IC, 3 * OC], F32R)
            wk2t = wp.tile([IC, 3 * OC], F32R)
            btile = wp.tile([OC, 1], F32)
            nc.sync.dma_start(out=wpt, in_=wpair)
            nc.sync.dma_start(out=wk2t, in_=wk2)
            nc.sync.dma_start(out=btile, in_=bias)

            def compute():
                for s in range(S):
                    for q in range(NQ):
                        # strip covers padded rows 56q .. 56q+58
                        strip = xp.tile([2 * IC, SROWS * WP], F32R, tag="strip")
                        sr = strip.rearrange("p (r c) -> p r c", c=WP)
                        # top half: padded rows, fully contiguous both sides
                        nc.scalar.dma_start(
                            out=sr[0:IC, :, :],
                            in_=xs[s, :, q * QROWS:q * QROWS + SROWS, :])
                        # bottom half = top shifted one row-slot down
                        nc.scalar.dma_start(
                            out=strip[IC:2 * IC, 0:(SROWS - 1) * WP],
                            in_=strip[0:IC, WP:SROWS * WP])

                        for g in range(QROWS // OBLK):
                            ot = op.tile([OC, OBLK, W], F32)
                            for bb in range(OBLK // BLK):
                                u = g * OBLK + bb * BLK
                                oh = q * QROWS + u
                                psum = pp.tile([OC, BLK, W], F32)
                                # slot u holds padded row 56q+u = input row
                                # 56q+u-1; pair mm at slots (u, u+1):
                                #   top    -> rows oh-1, oh   (kh=0)
                                #   bottom -> rows oh,   oh+1 (kh=1)
                                for i, kw in enumerate((0, 1, 2)):
                                    rhs = sr[:, u:u + BLK, kw:kw + W]
                                    nc.tensor.matmul(
                                        psum, wpt[:, kw * OC:(kw + 1) * OC], rhs,
                                        start=(i == 0), stop=False,
                                        skip_group_check=True)
                                # kh=2: input rows oh+1, oh+2 = slots u+2, u+3
                                for i, kw in enumerate((0, 1, 2)):
                                    rhs = sr[0:IC, u + 2:u + 2 + BLK, kw:kw + W]
                                    nc.tensor.matmul(
                                        psum, wk2t[:, kw * OC:(kw + 1) * OC], rhs,
                                        start=False, stop=(i == 2),
                                        skip_group_check=True)
                                nc.scalar.activation(
                                    ot[:, bb * BLK:(bb + 1) * BLK, :].rearrange(
                                        "p a b -> p (a b)"),
                                    psum.rearrange("p a b -> p (a b)"),
                                    mybir.ActivationFunctionType.Identity,
                                    bias=btile)
                            oh0 = q * QROWS + g * OBLK
                            nc.sync.dma_start(out=out[s, :, oh0:oh0 + OBLK, :], in_=ot)

            if repeat == 1:
                compute()
            else:
                with tc.For_i(0, repeat, 1):
                    compute()

    nc.compile()
    return nc


def host_prep(weight, bias):
    w = np.asarray(weight, dtype=np.float32)          # [oc, ic, kh, kw]
    wt = np.transpose(w, (1, 3, 0, 2))                # [ic, kw, oc, kh]
    wpair = np.concatenate([wt[:, :, :, 0], wt[:, :, :, 1]], axis=0) \
        .reshape(2 * IC, 3 * OC)
    wk2 = np.ascontiguousarray(wt[:, :, :, 2]).reshape(IC, 3 * OC)
    b = np.asarray(bias, dtype=np.float32).reshape(OC, 1)
    return wpair, wk2, b


def pad_x(x):
    xp_ = np.zeros((x.shape[0], IC, HP, WP), np.float32)
    xp_[:, :, 1:1 + H, 1:1 + W] = x
    return xp_


_module_cache = {}


def get_module(repeat=1):
    if repeat not in _module_cache:
        _module_cache[repeat] = build_module(repeat)
    return _module_cache[repeat]


def kernel(x, weight, bias):
    x = np.asarray(x, dtype=np.float32)
    wpair, wk2, b = host_prep(weight, bias)
    xp_ = pad_x(x)
    nc = get_module()
    in_maps = [{"xs": xp_[c * S:(c + 1) * S], "wpair": wpair, "wk2": wk2,
                "bias": b} for c in range(N_CORES)]
    res = run_bass_kernel_spmd(nc, in_maps, core_ids=list(range(N_CORES)))
    return np.concatenate([res.results[c]["out"] for c in range(N_CORES)], axis=0)



# revision 6
# speedup vs baseline: 1.7266x; 1.7266x over previous
"""Trainium2 Bass kernel for 3x3 conv (stride 1, pad 1) + bias.

x [32, 64, 224, 224] f32, weight [128, 64, 3, 3] f32, bias [128] f32
-> out [32, 128, 224, 224] f32.

Data-parallel over 8 NeuronCores: core c computes samples [4c, 4c+4).

Per-core scheme (v3, fp8 DoubleRow):
- Host splits x into x_hi = e4m3(x), x_lo = e4m3(x - x_hi) and stacks them
  on the channel dim: xhl [N, 128, 226, 226] e4m3 (padded). Partitions
  0-63 hold x_hi, 64-127 hold x_lo — no SBUF duplication DMAs at all.
- Weights: w_hi = e4m3(w), w_lo = e4m3(w - w_hi). For each of the 9
  (kh, kw) taps, one DoubleRow matmul accumulates
    t=0: [w_hi; w_hi] . [x_hi; x_lo]  = x * w_hi
    t=1: [w_lo; 0   ] . [x_hi; x_lo]  = x_hi * w_lo
  i.e. out = x*w_hi + x_hi*w_lo (the dropped x_lo*w_lo term is ~1e-6).
  Verified rel RMS vs fp32 reference: ~1.3e-3.
- DoubleRow runs 0.5 PE cycles/row: 9 DR matmuls * 224 cyc per 2-row
  block = 2016 cyc vs 2688 for the fp32r scheme, with half the input DMA.
- The rhs t-dim is a stride-0 broadcast of the same [128, 2, 224] window;
  both k-tiles read identical ifmap data against different weights.
- Whole sample (226*226 = 51 KB/partition fp8) resident in SBUF, double
  buffered; no strip overlap reloads.
- Loop order: tap i outer, PSUM bank inner (4 banks of 2 output rows) so
  the stationary weights can stay loaded across 4 matmuls.
- ScalarE evacuates psum->SBUF fused with bias add; 8-row store tiles.
  Loads ride the ACT HWDGE ring; stores alternate SP and DVE rings.
"""
import numpy as np
import ml_dtypes

import concourse.bass as bass
import concourse.mybir as mybir
import concourse.tile as tile
from concourse import bacc
from concourse.bass_utils import run_bass_kernel_spmd
from concourse._compat import axon_active

N_CORES = 8
S = 4                 # samples per core
IC, OC, H, W = 64, 128, 224, 224
HP, WP = H + 2, W + 2  # padded input dims (226)
GROWS = 8             # output rows per store tile / psum group
NG = H // GROWS       # 28 groups per sample
BLK = 2               # output rows per psum bank
NBANK = GROWS // BLK  # 4 banks per group

F8 = mybir.dt.float8e4
F32 = mybir.dt.float32
E4 = ml_dtypes.float8_e4m3


def build_module(repeat=1):
    nc = bacc.Bacc("TRN2", target_bir_lowering=False, debug=not axon_active(),
                   enable_asserts=True, num_devices=N_CORES)
    xs = nc.dram_tensor("xs", [S, 2 * IC, HP, WP], F8, kind="ExternalInput").ap()
    # wdr[p, i, t, oc]: tap i=(kh*3+kw); t=0 -> w_hi (both halves),
    # t=1 -> w_lo on p<64, zeros on p>=64
    wdr = nc.dram_tensor("wdr", [2 * IC, 9 * 2 * OC], F8, kind="ExternalInput").ap()
    bias = nc.dram_tensor("bias", [OC, 1], F32, kind="ExternalInput").ap()
    out = nc.dram_tensor("out", [S, OC, H, W], F32, kind="ExternalOutput").ap()

    with tile.TileContext(nc) as tc:
        with tc.tile_pool(name="wp", bufs=1) as wp, \
             tc.tile_pool(name="xp", bufs=2) as xp, \
             tc.tile_pool(name="op", bufs=3) as op, \
             tc.tile_pool(name="pp", bufs=2, space="PSUM") as pp:
            wt = wp.tile([2 * IC, 9, 2, OC], F8)
            btile = wp.tile([OC, 1], F32)
            nc.sync.dma_start(out=wt, in_=wdr.rearrange(
                "p (i t m) -> p i t m", i=9, t=2))
            nc.sync.dma_start(out=btile, in_=bias)

            def compute():
                for s in range(S):
                    xt = xp.tile([2 * IC, HP * WP], F8, tag="x")
                    nc.scalar.dma_start(
                        out=xt, in_=xs[s].rearrange("c h w -> c (h w)"))
                    xr = xt.rearrange("p (r c) -> p r c", c=WP)
                    for g in range(NG):
                        ot = op.tile([OC, GROWS, W], F32)
                        psums = [pp.tile([OC, BLK, W], F32, name=f"ps{bb}",
                                         tag=f"ps{bb}")
                                 for bb in range(NBANK)]
                        for kh in range(3):
                            for kw in range(3):
                                i = kh * 3 + kw
                                for bb in range(NBANK):
                                    u = g * GROWS + bb * BLK
                                    rhs = xr[:, u + kh:u + kh + BLK, kw:kw + W] \
                                        .unsqueeze(1).broadcast_to([2 * IC, 2, BLK, W])
                                    nc.tensor.matmul(
                                        psums[bb], wt[:, i], rhs,
                                        start=(i == 0), stop=(i == 8),
                                        perf_mode=mybir.MatmulPerfMode.DoubleRow,
                                        skip_group_check=True)
                        for bb in range(NBANK):
                            nc.scalar.activation(
                                ot[:, bb * BLK:(bb + 1) * BLK, :].rearrange(
                                    "p a b -> p (a b)"),
                                psums[bb].rearrange("p a b -> p (a b)"),
                                mybir.ActivationFunctionType.Identity,
                                bias=btile)
                        eng = nc.sync if g % 3 != 2 else nc.scalar
                        oh = g * GROWS
                        eng.dma_start(out=out[s, :, oh:oh + GROWS, :], in_=ot)

            if repeat == 1:
                compute()
            else:
                with tc.For_i(0, repeat, 1):
                    compute()

    nc.compile()
    return nc


def host_prep(weight, bias):
    w = np.asarray(weight, dtype=np.float32)          # [oc, ic, kh, kw]
    w_hi = w.astype(E4)
    w_lo = (w - w_hi.astype(np.float32)).astype(E4)
    # wdr[p, i, t, oc]
    wdr = np.zeros((2 * IC, 9, 2, OC), E4)
    wt_hi = np.transpose(w_hi, (1, 2, 3, 0))          # [ic, kh, kw, oc]
    wt_lo = np.transpose(w_lo, (1, 2, 3, 0))
    for khh in range(3):
        for kww in range(3):
            i = khh * 3 + kww
            wdr[:IC, i, 0, :] = wt_hi[:, khh, kww, :]
            wdr[IC:, i, 0, :] = wt_hi[:, khh, kww, :]
            wdr[:IC, i, 1, :] = wt_lo[:, khh, kww, :]
            # wdr[IC:, i, 1, :] stays 0
    b = np.asarray(bias, dtype=np.float32).reshape(OC, 1)
    return wdr.reshape(2 * IC, 9 * 2 * OC), b


def pad_x(x):
    """fp32 x [N, 64, 224, 224] -> padded hi/lo e4m3 [N, 128, 226, 226]."""
    x = np.asarray(x, dtype=np.float32)
    n = x.shape[0]
    xhl = np.zeros((n, 2 * IC, HP, WP), E4)
    xi = x.astype(E4)
    xhl[:, :IC, 1:1 + H, 1:1 + W] = xi
    xhl[:, IC:, 1:1 + H, 1:1 + W] = (x - xi.astype(np.float32)).astype(E4)
    return xhl


_module_cache = {}


def get_module(repeat=1):
    if repeat not in _module_cache:
        _module_cache[repeat] = build_module(repeat)
    return _module_cache[repeat]


def make_in_maps(x, weight, bias):
    wdr, b = host_prep(weight, bias)
    xhl = pad_x(x)
    return [{"xs": xhl[c * S:(c + 1) * S], "wdr": wdr, "bias": b}
            for c in range(N_CORES)]


def kernel(x, weight, bias):
    nc = get_module()
    in_maps = make_in_maps(x, weight, bias)
    res = run_bass_kernel_spmd(nc, in_maps, core_ids=list(range(N_CORES)))
    return np.concatenate([res.results[c]["out"] for c in range(N_CORES)], axis=0)
